# revision 2
# baseline (speedup 1.0000x reference)
"""Trainium2 Bass kernel for nn_DifferentiableRollout — z-space recursion.

Reference per step: x_{t+1} = x_t + DT*(tanh(xu_t @ W1 + b1) @ W2 + b2).

Rewrite the recursion in preactivation space z_t = xu_t @ W1 + b1:
    h_t = tanh(z_t)
    z_{t+1} = z_t + h_t @ W3 + du_t @ W1u + c        (W3 = DT*W2@W1x, rank<=64,
                                                      du_t = u_{t+1}-u_t,
                                                      c = DT*b2@W1x)
    dx_t = h_t @ (DT*W2)                             (x_{t+1} = x_t + dx_t + DT*b2)
z lives in PSUM fp32 forever; the critical chain per step is only
    tanh (ScalarE, PSUM->SBUF fp16) -> 16 fp16 matmuls (PE) -> tanh
with no DVE hop (the baseline's x-export stage is gone: x states are
recovered on the host by a cumulative sum over the exported dx_t).
This trades 3x more PE matmul work for a 2-stage chain; the PE stays
~100% busy which also keeps it at the full 2.4 GHz p-state.

Per core (128 batch rows), STREAMS=3 independent 43/43/42-wide streams
pipeline the chain. dx_t accumulates into a per-stream PSUM strip of
XGROUP=8 step slots; one DVE copy every 8 steps drains it to the fp16
output buffer (off the critical path).
"""

from contextlib import ExitStack

import numpy as np

import concourse.bacc as bacc
import concourse.bass as bass
import concourse.mybir as mybir
import concourse.tile as tile
from concourse.bass_utils import run_bass_kernel_spmd

B, T, SD, CD, H = 1024, 200, 64, 32, 512
DT = 0.1
NCORES = 8
STREAMS = 3
BLOCAL = B // NCORES
WIDTHS = [BLOCAL // STREAMS + (1 if s < BLOCAL % STREAMS else 0) for s in range(STREAMS)]
OFFS = [sum(WIDTHS[:s]) for s in range(STREAMS)]
HCH = H // 128                # 4 h-feature chunks
KU = CD + 1                   # 33: du rows + ones row (carries c)
XGROUP = 8                    # dx steps per PSUM strip / DVE drain
OUT_CHUNKS = 8

F16 = mybir.dt.float16
F32 = mybir.dt.float32


def _build_module(t_steps: int = T, streams: int = STREAMS, widths=None):
    if widths is None:
        widths = WIDTHS
    ts = bass.ts
    nc = bacc.Bacc(
        "TRN2",
        target_bir_lowering=False,
        debug=False,
        enable_asserts=False,
        num_devices=NCORES,
    )

    d_w1 = nc.dram_tensor("w1aug", [128, H], F16, kind="ExternalInput")
    d_w3 = nc.dram_tensor("w3blk", [128, HCH * HCH * 128], F16, kind="ExternalInput")
    d_wu = nc.dram_tensor("w1uaug", [KU, H], F16, kind="ExternalInput")
    d_w2 = nc.dram_tensor("w2pad", [128, HCH * SD], F16, kind="ExternalInput")
    d_x0, d_du, d_out = [], [], []
    for s in range(streams):
        w = widths[s]
        d_x0.append(nc.dram_tensor(f"x0aug{s}", [128, w], F16, kind="ExternalInput"))
        d_du.append(nc.dram_tensor(f"du{s}", [KU, t_steps * w], F16, kind="ExternalInput"))
        d_out.append(nc.dram_tensor(f"out{s}", [SD, t_steps * w], F16, kind="ExternalOutput"))

    with tile.TileContext(nc) as tc, ExitStack() as ctx:
        const = ctx.enter_context(tc.tile_pool(name="const", bufs=1))
        hpool = ctx.enter_context(tc.tile_pool(name="h", bufs=2))
        psum = ctx.enter_context(tc.tile_pool(name="psum", bufs=1, space="PSUM"))

        w1_sb = const.tile([128, H], F16)
        w3_sb = const.tile([128, HCH * HCH, 128], F16)
        wu_sb = const.tile([KU, H], F16)
        w2_sb = const.tile([128, HCH, SD], F16)

        psum_z, psum_dx, x0_sb, du_sb, out_sb = [], [], [], [], []
        for s in range(streams):
            w = widths[s]
            psum_z.append(psum.tile([128, HCH * w], F32, tag=f"pz{s}", name=f"pz{s}"))
            psum_dx.append(psum.tile([SD, XGROUP * w], F32, tag=f"pdx{s}", name=f"pdx{s}"))
            x0_sb.append(const.tile([128, w], F16, name=f"x0sb{s}"))
            du_sb.append(const.tile([KU, t_steps * w], F16, name=f"dusb{s}"))
            out_sb.append(const.tile([SD, t_steps * w], F16, name=f"outsb{s}"))

        # Startup DMAs in dependency-deadline order: x0 + first du chunk per
        # stream, then weights (w1 for the seed, w3 before step 0's mm3, w2
        # slightly later), then the remaining du round-robin.
        first = min(16, t_steps)
        for s in range(streams):
            w = widths[s]
            nc.sync.dma_start(x0_sb[s][:], d_x0[s].ap()[:])
            nc.gpsimd.dma_start(du_sb[s][:, 0 : first * w], d_du[s].ap()[:, 0 : first * w])
        nc.gpsimd.dma_start(w1_sb[:], d_w1.ap()[:])
        nc.sync.dma_start(w3_sb[:], d_w3.ap().rearrange("p (j c) -> p j c", j=HCH * HCH))
        nc.sync.dma_start(wu_sb[:], d_wu.ap()[:])
        nc.sync.dma_start(w2_sb[:], d_w2.ap().rearrange("p (j c) -> p j c", j=HCH))
        cb = np.linspace(first, t_steps, 5).astype(int)
        for k in range(len(cb) - 1):
            a, b = int(cb[k]), int(cb[k + 1])
            if a == b:
                continue
            for s in range(streams):
                w = widths[s]
                nc.sync.dma_start(
                    du_sb[s][:, a * w : b * w], d_du[s].ap()[:, a * w : b * w]
                )

        # Warm-up: a few matmuls + tanh overlap the preload DMAs so the ACT
        # tanh-table load (~2.7us) and PE pipeline spin-up are off step 0's
        # critical path. Targets psum_z[0]'s bank; the seed restarts it.
        warm_sb = const.tile([65, 65], F16, name="warm_sb")
        for _ in range(4):
            nc.tensor.matmul(
                psum_z[0][0:65, 0:65], w1_sb[0:65, 0:65], w1_sb[0:65, 0:65],
                start=True, stop=True, skip_group_check=True,
            )
        nc.scalar.activation(warm_sb[:], psum_z[0][0:65, 0:65],
                             mybir.ActivationFunctionType.Tanh)

        # Seed psum_z with z_0 = [x0; 1; u0; 0] @ W1aug. Only the first matmul
        # per bank uses start=True: start marks the whole 2KB zero-region
        # pending, so each later chunk's first write auto-overwrites.
        for s in range(streams):
            w = widths[s]
            for j in range(HCH):
                nc.tensor.matmul(
                    psum_z[s][:, ts(j, w)], w1_sb[:, ts(j, 128)], x0_sb[s][:],
                    start=j == 0, stop=False, skip_group_check=True,
                )

        for t in range(t_steps):
            for s in range(streams):
                w = widths[s]
                g = t % XGROUP
                h_sb = hpool.tile([128, HCH * w], F16, tag=f"h{s}", name=f"h{s}")
                nc.scalar.activation(
                    h_sb[:], psum_z[s][:], mybir.ActivationFunctionType.Tanh
                )
                if t < t_steps - 1:
                    # z += h @ W3 (16) then += [du;1] @ W1u_aug (4); the last
                    # writer gates the next tanh, so mm3 goes first.
                    for j in range(HCH):
                        for k in range(HCH):
                            nc.tensor.matmul(
                                psum_z[s][:, ts(j, w)],
                                w3_sb[:, k * HCH + j, :],
                                h_sb[:, ts(k, w)],
                                start=False, stop=False, skip_group_check=True,
                            )
                    last_z = t == t_steps - 2
                    for j in range(HCH):
                        nc.tensor.matmul(
                            psum_z[s][:, ts(j, w)],
                            wu_sb[:, ts(j, 128)],
                            du_sb[s][:, ts(t, w)],
                            start=False, stop=last_z and j == HCH - 1,
                            skip_group_check=True,
                        )
                # dx_t = h @ (DT*W2) into this group's PSUM strip slot. One
                # start per strip cycle (g==0): it marks the whole bank
                # pending-zero, so each later slot's first write overwrites.
                drain = g == XGROUP - 1 or t == t_steps - 1
                for k in range(HCH):
                    nc.tensor.matmul(
                        psum_dx[s][:, ts(g, w)],
                        w2_sb[:, k, :],
                        h_sb[:, ts(k, w)],
                        start=g == 0 and k == 0,
                        stop=drain and k == HCH - 1,
                        skip_group_check=True,
                    )
                if drain:
                    lo, n = t - g, g + 1
                    nc.vector.tensor_scalar_add(
                        out_sb[s][:, lo * w : (lo + n) * w],
                        psum_dx[s][:, 0 : n * w],
                        0.0,
                    )

        bounds = np.linspace(0, t_steps, OUT_CHUNKS + 1).astype(int)
        for s in range(streams):
            w = widths[s]
            for k in range(OUT_CHUNKS):
                a, b = int(bounds[k]), int(bounds[k + 1])
                if a == b:
                    continue
                nc.sync.dma_start(
                    d_out[s].ap()[:, a * w : b * w], out_sb[s][:, a * w : b * w]
                )

    nc.compile()
    return nc


_CACHE: dict = {}


def _get_module():
    if "nc" not in _CACHE:
        _CACHE["nc"] = _build_module()
    return _CACHE["nc"]


def _prep_inputs(x0, controls, W1, b1, W2, b2):
    """Host-side prep: shard, transpose, augment, cast. Returns in_maps."""
    f16 = np.float16
    W1 = np.asarray(W1, np.float32)
    b1 = np.asarray(b1, np.float32)
    W2 = np.asarray(W2, np.float32)
    b2 = np.asarray(b2, np.float32)
    x0 = np.asarray(x0, np.float32)
    controls = np.asarray(controls, np.float32)

    W1x, W1u = W1[:SD], W1[SD:]
    w1aug = np.concatenate(
        [W1x, b1[None, :], W1u, np.zeros((128 - SD - 1 - CD, H), np.float32)], axis=0
    ).astype(f16)
    W3 = DT * (W2 @ W1x)                                  # [H, H]
    w3blk = (
        W3.reshape(HCH, 128, HCH, 128)
        .transpose(0, 2, 1, 3)                            # [k, j, krow, jcol]
        .reshape(HCH * HCH, 128, 128)
        .transpose(1, 0, 2)                               # [krow, k*4+j, jcol]
        .reshape(128, HCH * HCH * 128)
    ).astype(f16)
    c_const = DT * (b2 @ W1x)                             # [H]
    w1uaug = np.concatenate([W1u, c_const[None, :]], axis=0).astype(f16)
    w2pad = (DT * W2).reshape(HCH, 128, SD).transpose(1, 0, 2).reshape(128, HCH * SD)
    w2pad = w2pad.astype(f16)

    x0T = x0.T.astype(f16)                                # [SD, B]
    u0T = controls[:, 0, :].T.astype(f16)                 # [CD, B]
    du = controls[:, 1:, :] - controls[:, :-1, :]         # [B, T-1, CD] fp32
    duT = du.transpose(1, 2, 0).astype(f16)               # [T-1, CD, B]

    in_maps = []
    for c in range(NCORES):
        m = {"w1aug": w1aug, "w3blk": w3blk, "w1uaug": w1uaug, "w2pad": w2pad}
        for s in range(STREAMS):
            w = WIDTHS[s]
            lo = c * BLOCAL + OFFS[s]
            cols = slice(lo, lo + w)
            m[f"x0aug{s}"] = np.concatenate(
                [
                    x0T[:, cols],
                    np.ones((1, w), f16),
                    u0T[:, cols],
                    np.zeros((128 - SD - 1 - CD, w), f16),
                ],
                axis=0,
            ).astype(f16)
            # [KU, T*w]: slot t holds [du_t; 1]; slot T-1 is unused zeros.
            ds = np.zeros((KU, T, w), f16)
            ds[:CD, : T - 1] = duT[:, :, cols].transpose(1, 0, 2)
            ds[CD, : T - 1] = 1.0
            m[f"du{s}"] = np.ascontiguousarray(ds).reshape(KU, T * w)
        in_maps.append(m)
    return in_maps


def kernel(x0, controls, W1, b1, W2, b2):
    nc = _get_module()
    in_maps = _prep_inputs(x0, controls, W1, b1, W2, b2)
    res = run_bass_kernel_spmd(nc, in_maps, core_ids=list(range(NCORES)))

    b2 = np.asarray(b2, np.float32)
    states = np.empty((B, T + 1, SD), np.float32)
    states[:, 0, :] = np.asarray(x0, np.float32)
    for c in range(NCORES):
        for s in range(STREAMS):
            w = WIDTHS[s]
            lo = c * BLOCAL + OFFS[s]
            out = np.asarray(res.results[c][f"out{s}"], np.float16)
            dx = out.reshape(SD, T, w).transpose(2, 1, 0).astype(np.float32)
            states[lo : lo + w, 1:, :] = np.cumsum(dx, axis=1)
    states[:, 1:, :] += states[:, :1, :] + DT * b2 * np.arange(1, T + 1, dtype=np.float32).reshape(1, T, 1)
    return states
